# revision 1
# baseline (speedup 1.0000x reference)
"""GATv2 message passing (nn_KG_GNN_84430467105347) on 8 Trainium2 NeuronCores.

Strategy (dst-sharded, no collectives):
  - Host: append self-loops, sort edges by destination, shard by dst range
    (12544 = 98*128 nodes per core). Each core owns the full softmax +
    aggregation for its dst range locally; per-core outputs are concatenated
    on the host. No inter-core communication.
  - Each core computes the full x_l = x@W_l+b_l table (projection) into its
    DRAM (fp16), plus x_r for its local nodes (kept in SBUF, fp16).
    Projections read a host-padded fp16 copy of x via HWDGE DMA-transpose
    (so the matmul lhsT needs no on-chip transposes).
  - Edge phase: for each 128-edge chunk (edges of one 128-dst-node tile):
    gather x_l[src] rows via indirect DMA, build a one-hot mask
    mask[e,d] = (dst_slot[e]==d) on the vector engine, expand x_r per edge
    with a PE matmul (+ x_l via identity matmul, accumulated in PSUM),
    LeakyReLU on the scalar engine, GATv2 scores via vector mul + segmented
    reduce, exp (no max subtraction -- score range [-7, 9], validated safe),
    and a single PE matmul per chunk accumulating both the softmax
    denominator and the weighted aggregation into PSUM
    ([128 dst, 128 feat | 4 denom]).
  - The static chunk schedule (chunks per node tile = max over cores) is
    computed from the actual edge data at kernel() time, so one program
    serves all 8 cores (SPMD); surplus chunks are padded (mask row = 0).
  - fp16 is used for the gather table / mask / aggregation rhs (2x DMA and
    2x DVE throughput); scores and all accumulations stay fp32. The exp
    values used for numerator and denominator are bitwise identical, so the
    softmax weights stay consistent; residual error ~1e-3 relative.
"""
import sys
sys.path.insert(0, '/opt/trn_rl_repo')
import numpy as np

N_NODES = 100000
IN_DIM = 128
H, C = 4, 32
F = 128           # = H*C = IN_DIM
P = 128
NEG_SLOPE = 0.2
N_CORES = 8
NT = 98           # node tiles per core
NPC = NT * P      # 12544 nodes per core (padded; 8*12544 = 100352 >= 100000)
NPAD = N_CORES * NPC
GB = 8            # chunks per batch group
PB = 4            # projection node-tiles per iteration
EDT_NP = np.float16   # edge-pipeline dtype (np.float32 for exact fallback)


def _host_prep(src, dst):
    """Sort by dst, shard by dst range, build per-core static chunk layout."""
    N = N_NODES
    s = np.concatenate([np.asarray(src, dtype=np.int64),
                        np.arange(N, dtype=np.int64)])
    d = np.concatenate([np.asarray(dst, dtype=np.int64),
                        np.arange(N, dtype=np.int64)])
    order = np.argsort(d, kind='stable')
    s = s[order].astype(np.int32)
    d = d[order].astype(np.int32)
    core = d // NPC
    tile_of = (d % NPC) // P
    slot_of = d % P
    counts = np.zeros((N_CORES, NT), dtype=np.int64)
    np.add.at(counts, (core, tile_of), 1)
    cpt = np.maximum(1, -(-counts.max(axis=0) // P))      # chunks per tile
    nchunk = int(cpt.sum())
    pad_groups = (-nchunk) % GB
    cpt[NT - 1] += pad_groups                              # pad to multiple of GB
    nchunk += pad_groups
    cbase = np.zeros(NT + 1, dtype=np.int64)
    np.cumsum(cpt, out=cbase[1:])

    src_T = np.zeros((N_CORES, P, nchunk), dtype=np.int32)
    dst_T = np.full((N_CORES, P, nchunk), 255.0, dtype=np.float32)
    core_starts = np.searchsorted(core, np.arange(N_CORES + 1))
    for k in range(N_CORES):
        lo, hi = core_starts[k], core_starts[k + 1]
        sk, tk, slk = s[lo:hi], tile_of[lo:hi], slot_of[lo:hi]
        tile_starts = np.searchsorted(tk, np.arange(NT + 1))
        for t in range(NT):
            a, b = tile_starts[t], tile_starts[t + 1]
            n = b - a
            if n == 0:
                continue
            pos = cbase[t] * P + np.arange(n)              # linear slot in capacity
            ch = pos // P
            pp = pos % P
            src_T[k, pp, ch] = sk[a:b]
            dst_T[k, pp, ch] = slk[a:b].astype(np.float32)
    return src_T, dst_T, cpt, cbase, nchunk


def _build_program(nchunk, cpt, cbase, edt_np, repeat_edge=1, repeat_proj=1):
    import concourse.bass as bass
    import concourse.mybir as mybir
    import concourse.tile as tile
    from concourse import bacc
    from concourse.masks import make_identity

    edt = mybir.dt.float32 if edt_np == np.float32 else mybir.dt.float16
    f32 = mybir.dt.float32
    NXT = NPAD // P                                         # 784 projection tiles
    assert NXT % PB == 0

    nc = bacc.Bacc(None, target_bir_lowering=False)
    x16_in = nc.dram_tensor("x16", [NPAD, IN_DIM], edt, kind="ExternalInput")
    xloc_in = nc.dram_tensor("xloc", [NPC, IN_DIM], edt, kind="ExternalInput")
    wlr_in = nc.dram_tensor("wlr", [IN_DIM, 2 * F], edt, kind="ExternalInput")
    blr_in = nc.dram_tensor("blr", [1, 2 * F], edt, kind="ExternalInput")
    att_big_in = nc.dram_tensor("att_big", [P, GB * F], f32, kind="ExternalInput")
    bias_b_in = nc.dram_tensor("bias_b", [P, F], f32, kind="ExternalInput")
    iota_in = nc.dram_tensor("iota_row", [P, P], f32, kind="ExternalInput")
    ones_in = nc.dram_tensor("ones_row", [1, P], edt, kind="ExternalInput")
    srcT_in = nc.dram_tensor("srcT", [P, nchunk], mybir.dt.int32, kind="ExternalInput")
    dstT_in = nc.dram_tensor("dstT", [P, nchunk], f32, kind="ExternalInput")
    out_dram = nc.dram_tensor("out", [NPC, F], f32, kind="ExternalOutput")

    with tile.TileContext(nc) as tc:
        with tc.tile_pool(name="persist", bufs=1) as pp, \
             tc.tile_pool(name="dram", bufs=1, space="DRAM") as dramp:
            xl_dram = dramp.tile([NPAD, F], edt)
            ident16 = pp.tile([P, P], edt)
            make_identity(nc, ident16[:])
            iota_row = pp.tile([P, P], f32)
            nc.sync.dma_start(iota_row[:], iota_in[:])
            att_big = pp.tile([P, GB * F], f32)
            nc.sync.dma_start(att_big[:], att_big_in[:])
            bias_b = pp.tile([P, F], f32)
            nc.sync.dma_start(bias_b[:], bias_b_in[:])
            wlr = pp.tile([IN_DIM, 2 * F], edt)
            nc.sync.dma_start(wlr[:], wlr_in[:])
            blr = pp.tile([1, 2 * F], edt)
            nc.sync.dma_start(blr[:], blr_in[:])
            ones_row = pp.tile([1, P], edt)
            nc.sync.dma_start(ones_row[:], ones_in[:])
            srcT = pp.tile([P, nchunk], mybir.dt.int32)
            nc.sync.dma_start(srcT[:], srcT_in[:])
            dstT = pp.tile([P, nchunk], f32)
            nc.sync.dma_start(dstT[:], dstT_in[:])
            xr_all = pp.tile([P, NT, F], edt)

            # ---------------- projection phase ----------------
            # x_l for all (padded) nodes -> xl_dram; x_r for local nodes -> SBUF
            with tc.tile_pool(name="proj_sb", bufs=3) as sb, \
                 tc.tile_pool(name="proj_ps", bufs=2, space="PSUM") as ps:
              for _rep in range(repeat_proj):
                for i in range(NXT // PB):
                    r0 = i * PB * P
                    xT = sb.tile([P, PB * P], edt, tag="xT")
                    nc.sync.dma_start(xT[:], x16_in[r0:r0 + PB * P, :],
                                      transpose=True)
                    prj = ps.tile([P, PB, 2 * F], f32, tag="prj", space="PSUM")
                    for j in range(PB):
                        nc.tensor.matmul(out=prj[:, j, :],
                                         lhsT=xT[:, j * P:(j + 1) * P],
                                         rhs=wlr[:], start=True, stop=False)
                        nc.tensor.matmul(out=prj[:, j, :], lhsT=ones_row[:],
                                         rhs=blr[:], start=False, stop=True)
                    xl_t = sb.tile([P, PB, F], edt, tag="xl")
                    nc.scalar.copy(xl_t[:], prj[:, :, :F])
                    nc.sync.dma_start(
                        out=xl_dram[r0:r0 + PB * P, :].rearrange(
                            "(b p) f -> p b f", p=P),
                        in_=xl_t[:])
                # local x_r tiles from the per-core xloc input
                for i in range(NT // PB + 1):
                    t0 = i * PB
                    nb = min(PB, NT - t0)
                    if nb <= 0:
                        break
                    r0 = t0 * P
                    xT = sb.tile([P, PB * P], edt, tag="xT")
                    nc.sync.dma_start(xT[:, :nb * P],
                                      xloc_in[r0:r0 + nb * P, :], transpose=True)
                    prj = ps.tile([P, PB, F], f32, tag="prjr", space="PSUM")
                    for j in range(nb):
                        nc.tensor.matmul(out=prj[:, j, :],
                                         lhsT=xT[:, j * P:(j + 1) * P],
                                         rhs=wlr[:, F:], start=True, stop=False)
                        nc.tensor.matmul(out=prj[:, j, :], lhsT=ones_row[:],
                                         rhs=blr[:, F:], start=False, stop=True)
                    nc.scalar.copy(xr_all[:, t0:t0 + nb, :], prj[:, :nb, :])

            # ---------------- edge phase ----------------
            with tc.tile_pool(name="eg_sb", bufs=3) as sb, \
                 tc.tile_pool(name="eg_ps", bufs=2, space="PSUM") as ps, \
                 tc.tile_pool(name="eg_ps1", bufs=2, space="PSUM") as ps1, \
                 tc.tile_pool(name="out_sb", bufs=3) as osb:
                ngroups = nchunk // GB
                t_of = np.zeros(nchunk, dtype=np.int64)
                for t in range(NT):
                    t_of[cbase[t]:cbase[t + 1]] = t
                out_ps = None
                for _rep in range(repeat_edge):
                  for g in range(ngroups):
                    g_t = sb.tile([P, GB, F], edt, tag="g")
                    mask = sb.tile([P, GB, P], edt, tag="mask")
                    mt_ps = ps1.tile([P, GB, P], edt, tag="mt", space="PSUM")
                    maskT = sb.tile([P, GB, P], edt, tag="maskT")
                    m_ps = ps.tile([P, GB, F], f32, tag="m", space="PSUM")
                    m_t = sb.tile([P, GB, F], f32, tag="mt_sb")
                    mw = sb.tile([P, GB, F], f32, tag="mw")
                    rhsw = sb.tile([P, GB, F + H], edt, tag="rhsw")
                    esc32 = sb.tile([P, GB, H], f32, tag="esc")
                    for b in range(GB):
                        c = g * GB + b
                        nc.gpsimd.indirect_dma_start(
                            out=g_t[:, b, :], out_offset=None, in_=xl_dram[:],
                            in_offset=bass.IndirectOffsetOnAxis(
                                ap=srcT[:, c:c + 1], axis=0))
                        nc.vector.tensor_tensor(
                            out=mask[:, b, :],
                            in0=dstT[:, c:c + 1].to_broadcast([P, P]),
                            in1=iota_row[:],
                            op=mybir.AluOpType.is_equal)
                        nc.tensor.transpose(out=mt_ps[:, b, :], in_=mask[:, b, :],
                                            identity=ident16[:])
                    nc.scalar.copy(maskT[:], mt_ps[:])
                    for b in range(GB):
                        c = g * GB + b
                        t = int(t_of[c])
                        nc.tensor.matmul(out=m_ps[:, b, :], lhsT=maskT[:, b, :],
                                         rhs=xr_all[:, t, :], start=True, stop=False)
                        nc.tensor.matmul(out=m_ps[:, b, :], lhsT=ident16[:],
                                         rhs=g_t[:, b, :], start=False, stop=True)
                    nc.scalar.activation(out=m_t[:], in_=m_ps[:],
                                         func=mybir.ActivationFunctionType.Prelu,
                                         alpha=NEG_SLOPE)
                    nc.vector.tensor_tensor(
                        out=mw[:].rearrange("p b f -> p (b f)"),
                        in0=m_t[:].rearrange("p b f -> p (b f)"),
                        in1=att_big[:],
                        op=mybir.AluOpType.mult)
                    nc.vector.tensor_reduce(
                        out=esc32[:], in_=mw[:].rearrange("p b (h c) -> p (b h) c", h=H),
                        axis=mybir.AxisListType.X, op=mybir.AluOpType.add)
                    nc.scalar.activation(out=rhsw[:, :, F:], in_=esc32[:],
                                         func=mybir.ActivationFunctionType.Exp)
                    nc.vector.tensor_tensor(
                        out=rhsw[:, :, :F].rearrange("p b (h c) -> p b h c", h=H),
                        in0=g_t[:].rearrange("p b (h c) -> p b h c", h=H),
                        in1=rhsw[:, :, F:][:, :, :, None].to_broadcast([P, GB, H, C]),
                        op=mybir.AluOpType.mult)
                    for b in range(GB):
                        c = g * GB + b
                        t = int(t_of[c])
                        if c == cbase[t]:
                            out_ps = ps.tile([P, F + H], f32, tag="out", space="PSUM")
                        nc.tensor.matmul(out=out_ps[:], lhsT=mask[:, b, :],
                                         rhs=rhsw[:, b, :],
                                         start=(c == cbase[t]),
                                         stop=(c == cbase[t + 1] - 1))
                        if c == cbase[t + 1] - 1:
                            den = osb.tile([P, H], f32, tag="den")
                            nc.vector.tensor_scalar_max(den[:], out_ps[:, F:], 1e-30)
                            recip = osb.tile([P, H], f32, tag="recip")
                            nc.vector.reciprocal(recip[:], den[:])
                            fin = osb.tile([P, F], f32, tag="fin")
                            nc.vector.tensor_tensor(
                                out=fin[:].rearrange("p (h c) -> p h c", h=H),
                                in0=out_ps[:, :F].rearrange("p (h c) -> p h c", h=H),
                                in1=recip[:, :, None].to_broadcast([P, H, C]),
                                op=mybir.AluOpType.mult)
                            fin2 = osb.tile([P, F], f32, tag="fin2")
                            nc.vector.tensor_tensor(
                                out=fin2[:], in0=fin[:], in1=bias_b[:],
                                op=mybir.AluOpType.add)
                            nc.sync.dma_start(out_dram[t * P:(t + 1) * P, :], fin2[:])
    nc.compile()
    return nc


def _make_in_maps(x, W_l, b_l, W_r, b_r, att, bias, src_T, dst_T):
    edt = EDT_NP
    wlr = np.concatenate([W_l, W_r], axis=1).astype(edt)         # [128, 256]
    blr = np.concatenate([b_l, b_r])[None, :].astype(edt)        # [1, 256]
    att_big = np.tile(att.reshape(1, F), (P, GB)).astype(np.float32)
    bias_b = np.tile(bias[None, :], (P, 1)).astype(np.float32)
    iota = np.tile(np.arange(P, dtype=np.float32)[None, :], (P, 1))
    ones_row = np.ones((1, P), dtype=edt)
    x_pad = np.zeros((NPAD, IN_DIM), dtype=edt)
    x_pad[:N_NODES] = x.astype(edt)
    in_maps = []
    for k in range(N_CORES):
        in_maps.append({
            "x16": x_pad, "xloc": np.ascontiguousarray(
                x_pad[k * NPC:(k + 1) * NPC]),
            "wlr": wlr, "blr": blr, "att_big": att_big, "bias_b": bias_b,
            "iota_row": iota, "ones_row": ones_row,
            "srcT": src_T[k], "dstT": dst_T[k],
        })
    return in_maps


def kernel(x, W_l, b_l, W_r, b_r, att, bias, src, dst):
    x = np.asarray(x, dtype=np.float32)
    W_l = np.asarray(W_l, dtype=np.float32)
    W_r = np.asarray(W_r, dtype=np.float32)
    b_l = np.asarray(b_l, dtype=np.float32)
    b_r = np.asarray(b_r, dtype=np.float32)
    att = np.asarray(att, dtype=np.float32)
    bias = np.asarray(bias, dtype=np.float32)

    src_T, dst_T, cpt, cbase, nchunk = _host_prep(src, dst)
    nc = _build_program(nchunk, cpt, cbase, EDT_NP)
    in_maps = _make_in_maps(x, W_l, b_l, W_r, b_r, att, bias, src_T, dst_T)

    from concourse import bass2jax
    results = bass2jax.run_bass_via_pjrt(nc, in_maps, n_cores=N_CORES)

    out = np.empty((N_NODES, F), dtype=np.float32)
    for k in range(N_CORES):
        lo = k * NPC
        hi = min(lo + NPC, N_NODES)
        out[lo:hi] = results[k]["out"][:hi - lo]
    return out



# revision 2
# speedup vs baseline: 1.3882x; 1.3882x over previous
"""GATv2 message passing (nn_KG_GNN_84430467105347) on 8 Trainium2 NeuronCores.

v3 (dst-sharded, slot-aligned edges, no collectives):
  - Host: self-loops handled as a dedicated first chunk per tile (served from
    SBUF, no gather). Remaining edges dealt to (tile, slot) positions in
    descending in-degree order (~1% padding). Edge (p, c): partition p = dst
    slot.
  - Phase 1: xl table (fp16) for all nodes -> DRAM from host-pretransposed xT;
    poison row at index NPAD kills padded edges' scores (exp -> 0).
    xr + xl for the core's local nodes -> SBUF.
  - Phase 2 per chunk: indirect-DMA gather of xl[src] rows (slot-aligned);
    m = g + xr[tile] on vector; Prelu on scalar; scores = reduce(m*att) on
    vector; exp on scalar; payload = g * p on vector; scatter-add is an
    identity-stationary matmul accumulating [128 dst x (F num | H den)] in
    PSUM. Tile end: divide, add bias, DMA out. Host unpermutes rows.
"""
import sys
sys.path.insert(0, '/opt/trn_rl_repo')
import numpy as np

N_NODES = 100000
IN_DIM = 128
H, C = 4, 32
F = 128
P = 128
NEG_SLOPE = 0.2
N_CORES = 8
NT = 98
NPC = NT * P
NPAD = N_CORES * NPC
NXT = NPAD // P
B = 8             # chunks per edge-phase group
PB = 8            # projection tiles per iteration
POISON_K = 30.0


def _host_prep(src, dst):
    """Self-loop chunk per tile + degree-balanced slot layout for real edges."""
    N = N_NODES
    s = np.concatenate([np.asarray(src, dtype=np.int64)]).astype(np.int32)
    d = np.concatenate([np.asarray(dst, dtype=np.int64)]).astype(np.int32)
    deg = np.bincount(d, minlength=N).astype(np.int64)   # excl self loops

    perm = np.full((N_CORES, NPC), -1, dtype=np.int64)
    tile_of = np.zeros(N, dtype=np.int32)
    slot_of = np.zeros(N, dtype=np.int32)
    cpt_k = np.zeros((N_CORES, NT), dtype=np.int64)
    for k in range(N_CORES):
        lo = k * NPC
        hi = min(lo + NPC, N)
        nodes = np.arange(lo, hi, dtype=np.int64)
        order = np.argsort(-deg[nodes], kind='stable')
        pn = nodes[order]
        perm[k, :len(pn)] = pn
        r = np.arange(len(pn))
        tile_of[pn] = r // P
        slot_of[pn] = r % P
        dsorted = np.zeros(NPC, dtype=np.int64)
        dsorted[:len(pn)] = deg[pn]
        cpt_k[k] = dsorted[::P][:NT]

    cpt = cpt_k.max(axis=0) + 1                     # +1 = self-loop chunk
    nchunk = int(cpt.sum())
    pad = (-nchunk) % B
    cpt[NT - 1] += pad
    nchunk += pad
    cbase = np.zeros(NT + 1, dtype=np.int64)
    np.cumsum(cpt, out=cbase[1:])

    srcT = np.full((N_CORES, P, nchunk), NPAD, dtype=np.int32)  # default: poison
    core = d // NPC
    for k in range(N_CORES):
        m = core == k
        dn, sn = d[m], s[m]
        key = tile_of[dn].astype(np.int64) * P + slot_of[dn]
        o2 = np.argsort(key, kind='stable')
        key_s, sn_s = key[o2], sn[o2]
        first = np.searchsorted(key_s, np.arange(NPC))
        occ = np.arange(len(key_s)) - first[key_s]
        chunkpos = cbase[key_s // P] + 1 + occ      # +1 skips self chunk
        srcT[k, key_s % P, chunkpos] = sn_s
    return srcT, cpt, cbase, nchunk, perm


def _build_program(nchunk, cpt, cbase):
    import concourse.bass as bass
    import concourse.mybir as mybir
    import concourse.tile as tile
    from concourse import bacc
    from concourse.masks import make_identity

    f16 = mybir.dt.float16
    f32 = mybir.dt.float32

    t_of = np.zeros(nchunk, dtype=np.int64)
    for t in range(NT):
        t_of[cbase[t]:cbase[t + 1]] = t
    is_self = np.zeros(nchunk, dtype=bool)
    is_self[cbase[:NT]] = True

    nc = bacc.Bacc(None, target_bir_lowering=False)
    xT_in = nc.dram_tensor("xT", [P, NPAD], f16, kind="ExternalInput")
    xlocT_in = nc.dram_tensor("xlocT", [P, NPC], f16, kind="ExternalInput")
    wl_in = nc.dram_tensor("wl", [IN_DIM, F], f16, kind="ExternalInput")
    wr_in = nc.dram_tensor("wr", [IN_DIM, F], f16, kind="ExternalInput")
    blrb_in = nc.dram_tensor("blrb", [P, 2 * F], f16, kind="ExternalInput")
    att_big_in = nc.dram_tensor("att_big", [P, B * F], f16, kind="ExternalInput")
    bias_b_in = nc.dram_tensor("bias_b", [P, F], f32, kind="ExternalInput")
    poison_in = nc.dram_tensor("poison", [1, F], f16, kind="ExternalInput")
    srcT_in = nc.dram_tensor("srcT", [P, nchunk], mybir.dt.int32, kind="ExternalInput")
    out_dram = nc.dram_tensor("out", [NPC, F], f32, kind="ExternalOutput")

    with tile.TileContext(nc) as tc:
        with tc.tile_pool(name="persist", bufs=1) as pp, \
             tc.tile_pool(name="dram", bufs=1, space="DRAM") as dramp:
            xl_dram = dramp.tile([NPAD + 1, F], f16)
            ident = pp.tile([P, P], f16)
            make_identity(nc, ident[:])
            wl = pp.tile([IN_DIM, F], f16)
            nc.sync.dma_start(wl[:], wl_in[:])
            wr = pp.tile([IN_DIM, F], f16)
            nc.sync.dma_start(wr[:], wr_in[:])
            blrb = pp.tile([P, 2 * F], f16)
            nc.sync.dma_start(blrb[:], blrb_in[:])
            att_big = pp.tile([P, B * F], f16)
            nc.sync.dma_start(att_big[:], att_big_in[:])
            bias_b = pp.tile([P, F], f32)
            nc.sync.dma_start(bias_b[:], bias_b_in[:])
            srcT = pp.tile([P, nchunk], mybir.dt.int32)
            nc.sync.dma_start(srcT[:], srcT_in[:])
            nc.sync.dma_start(xl_dram[NPAD:NPAD + 1, :], poison_in[:])
            xr_all = pp.tile([P, NT, F], f16)
            xl_self = pp.tile([P, NT, F], f16)

            # ---------------- phase 1: projections ----------------
            with tc.tile_pool(name="proj_sb", bufs=3) as sb, \
                 tc.tile_pool(name="proj_ps", bufs=2, space="PSUM") as ps:
                for i in range(NXT // PB):
                    r0 = i * PB * P
                    xTt = sb.tile([P, PB * P], f16, tag="xTt")
                    nc.sync.dma_start(xTt[:], xT_in[:, r0:r0 + PB * P])
                    prj = ps.tile([P, PB, F], f32, tag="prj", space="PSUM")
                    for j in range(PB):
                        nc.tensor.matmul(out=prj[:, j, :],
                                         lhsT=xTt[:, j * P:(j + 1) * P],
                                         rhs=wl[:], start=True, stop=True)
                    xl_t = sb.tile([P, PB, F], f16, tag="xl_t")
                    nc.vector.tensor_tensor(
                        out=xl_t[:], in0=prj[:],
                        in1=blrb[:, None, :F].to_broadcast([P, PB, F]),
                        op=mybir.AluOpType.add)
                    nc.sync.dma_start(
                        out=xl_dram[r0:r0 + PB * P, :].rearrange(
                            "(b p) f -> p b f", p=P),
                        in_=xl_t[:])
                # local tiles: xr and xl_self from xlocT
                for i in range((NT + PB - 1) // PB):
                    t0 = i * PB
                    nb = min(PB, NT - t0)
                    r0 = t0 * P
                    xTt = sb.tile([P, PB * P], f16, tag="xTt")
                    nc.sync.dma_start(xTt[:, :nb * P], xlocT_in[:, r0:r0 + nb * P])
                    prj = ps.tile([P, PB, F], f32, tag="prj", space="PSUM")
                    prl = ps.tile([P, PB, F], f32, tag="prl", space="PSUM")
                    for j in range(nb):
                        nc.tensor.matmul(out=prj[:, j, :],
                                         lhsT=xTt[:, j * P:(j + 1) * P],
                                         rhs=wr[:], start=True, stop=True)
                        nc.tensor.matmul(out=prl[:, j, :],
                                         lhsT=xTt[:, j * P:(j + 1) * P],
                                         rhs=wl[:], start=True, stop=True)
                    nc.vector.tensor_tensor(
                        out=xr_all[:, t0:t0 + nb, :], in0=prj[:, :nb, :],
                        in1=blrb[:, None, F:].to_broadcast([P, nb, F]),
                        op=mybir.AluOpType.add)
                    nc.vector.tensor_tensor(
                        out=xl_self[:, t0:t0 + nb, :], in0=prl[:, :nb, :],
                        in1=blrb[:, None, :F].to_broadcast([P, nb, F]),
                        op=mybir.AluOpType.add)

            # ---------------- phase 2: edges ----------------
            with tc.tile_pool(name="eg_sb", bufs=4) as sb, \
                 tc.tile_pool(name="out_ps", bufs=2, space="PSUM") as ops, \
                 tc.tile_pool(name="out_sb", bufs=3) as osb:
                ngroups = nchunk // B
                out_ps = None
                for g in range(ngroups):
                    c0 = g * B
                    g_t = sb.tile([P, B, F], f16, tag="g")
                    for j in range(B):
                        c = c0 + j
                        if is_self[c]:
                            continue
                        nc.gpsimd.indirect_dma_start(
                            out=g_t[:, j, :], out_offset=None, in_=xl_dram[:],
                            in_offset=bass.IndirectOffsetOnAxis(
                                ap=srcT[:, c:c + 1], axis=0))
                    # m = xl[src] + xr[dst tile]  (vector; runs of equal tile)
                    m_sb = sb.tile([P, B, F], f16, tag="m")
                    j = 0
                    while j < B:
                        c = c0 + j
                        t = int(t_of[c])
                        if is_self[c]:
                            nc.vector.tensor_tensor(
                                out=m_sb[:, j, :], in0=xl_self[:, t, :],
                                in1=xr_all[:, t, :], op=mybir.AluOpType.add)
                            j += 1
                            continue
                        j1 = j
                        while (j1 < B and int(t_of[c0 + j1]) == t
                               and not is_self[c0 + j1]):
                            j1 += 1
                        nc.vector.tensor_tensor(
                            out=m_sb[:, j:j1, :], in0=g_t[:, j:j1, :],
                            in1=xr_all[:, t, None, :].to_broadcast([P, j1 - j, F]),
                            op=mybir.AluOpType.add)
                        j = j1
                    # LeakyReLU
                    mp16 = sb.tile([P, B, F], f16, tag="mp")
                    nc.scalar.activation(out=mp16[:].rearrange("p b f -> p (b f)"),
                                         in_=m_sb[:].rearrange("p b f -> p (b f)"),
                                         func=mybir.ActivationFunctionType.Prelu,
                                         alpha=NEG_SLOPE)
                    # scores
                    mw = sb.tile([P, B, F], f16, tag="mw")
                    nc.vector.tensor_tensor(
                        out=mw[:].rearrange("p b f -> p (b f)"),
                        in0=mp16[:].rearrange("p b f -> p (b f)"),
                        in1=att_big[:], op=mybir.AluOpType.mult)
                    esc = sb.tile([P, B, H], f32, tag="esc")
                    nc.vector.tensor_reduce(
                        out=esc[:].rearrange("p b h -> p (b h)"),
                        in_=mw[:].rearrange("p b (h c) -> p (b h) c", h=H),
                        axis=mybir.AxisListType.X, op=mybir.AluOpType.add)
                    rhsw = sb.tile([P, B, F + H], f16, tag="rhsw")
                    nc.scalar.activation(out=rhsw[:, :, F:], in_=esc[:],
                                         func=mybir.ActivationFunctionType.Exp)
                    # weighted payload (self chunks use xl_self)
                    j = 0
                    while j < B:
                        c = c0 + j
                        t = int(t_of[c])
                        if is_self[c]:
                            nc.vector.tensor_tensor(
                                out=rhsw[:, j, :F].rearrange(
                                    "p (h c) -> p h c", h=H),
                                in0=xl_self[:, t, :].rearrange(
                                    "p (h c) -> p h c", h=H),
                                in1=rhsw[:, j, F:][:, :, None].to_broadcast(
                                    [P, H, C]),
                                op=mybir.AluOpType.mult)
                            j += 1
                            continue
                        j1 = j
                        while (j1 < B and not is_self[c0 + j1]):
                            j1 += 1
                        nc.vector.tensor_tensor(
                            out=rhsw[:, j:j1, :F].rearrange(
                                "p b (h c) -> p b h c", h=H),
                            in0=g_t[:, j:j1, :].rearrange(
                                "p b (h c) -> p b h c", h=H),
                            in1=rhsw[:, j:j1, F:][:, :, :, None].to_broadcast(
                                [P, j1 - j, H, C]),
                            op=mybir.AluOpType.mult)
                        j = j1
                    # scatter accumulation (identity stationary)
                    for j in range(B):
                        c = c0 + j
                        t = int(t_of[c])
                        if c == cbase[t]:
                            out_ps = ops.tile([P, F + H], f32, tag="out",
                                              space="PSUM")
                        nc.tensor.matmul(out=out_ps[:], lhsT=ident[:],
                                         rhs=rhsw[:, j, :],
                                         start=(c == cbase[t]),
                                         stop=(c == cbase[t + 1] - 1))
                        if c == cbase[t + 1] - 1:
                            den = osb.tile([P, H], f32, tag="den")
                            nc.vector.tensor_scalar_max(den[:], out_ps[:, F:],
                                                        1e-30)
                            recip = osb.tile([P, H], f32, tag="recip")
                            nc.vector.reciprocal(recip[:], den[:])
                            fin = osb.tile([P, F], f32, tag="fin")
                            nc.vector.tensor_tensor(
                                out=fin[:].rearrange("p (h c) -> p h c", h=H),
                                in0=out_ps[:, :F].rearrange(
                                    "p (h c) -> p h c", h=H),
                                in1=recip[:, :, None].to_broadcast([P, H, C]),
                                op=mybir.AluOpType.mult)
                            fin2 = osb.tile([P, F], f32, tag="fin2")
                            nc.vector.tensor_tensor(
                                out=fin2[:], in0=fin[:], in1=bias_b[:],
                                op=mybir.AluOpType.add)
                            nc.sync.dma_start(out_dram[t * P:(t + 1) * P, :],
                                              fin2[:])
    nc.compile()
    return nc


def _make_in_maps(x, W_l, b_l, W_r, b_r, att, bias, srcT, perm):
    f16 = np.float16
    x_pad = np.zeros((NPAD, IN_DIM), dtype=f16)
    x_pad[:N_NODES] = x.astype(f16)
    xT = np.ascontiguousarray(x_pad.T)
    wl = W_l.astype(f16)
    wr = W_r.astype(f16)
    blrb = np.tile(np.concatenate([b_l, b_r])[None, :], (P, 1)).astype(f16)
    att_big = np.tile(att.reshape(1, F), (P, B)).astype(f16)
    bias_b = np.tile(bias[None, :], (P, 1)).astype(np.float32)
    poison = (-np.sign(att.reshape(1, F)) * POISON_K).astype(f16)
    poison[poison == 0] = POISON_K
    in_maps = []
    for k in range(N_CORES):
        pk = np.maximum(perm[k], 0)
        xlocT = np.ascontiguousarray(x_pad[pk].T)
        in_maps.append({
            "xT": xT, "xlocT": xlocT, "wl": wl, "wr": wr, "blrb": blrb,
            "att_big": att_big, "bias_b": bias_b, "poison": poison,
            "srcT": srcT[k],
        })
    return in_maps


def build_for_bench(inp):
    x = np.asarray(inp['x'], dtype=np.float32)
    srcT, cpt, cbase, nchunk, perm = _host_prep(inp['src'], inp['dst'])
    nc = _build_program(nchunk, cpt, cbase)
    in_maps = _make_in_maps(x, inp['W_l'], inp['b_l'], inp['W_r'], inp['b_r'],
                            inp['att'], inp['bias'], srcT, perm)

    def post(results):
        out = np.empty((N_NODES, F), dtype=np.float32)
        for k in range(N_CORES):
            pk = perm[k]
            real = pk >= 0
            out[pk[real]] = results[k]["out"][real]
        return out
    return nc, in_maps, post


def kernel(x, W_l, b_l, W_r, b_r, att, bias, src, dst):
    x = np.asarray(x, dtype=np.float32)
    W_l = np.asarray(W_l, dtype=np.float32)
    W_r = np.asarray(W_r, dtype=np.float32)
    b_l = np.asarray(b_l, dtype=np.float32)
    b_r = np.asarray(b_r, dtype=np.float32)
    att = np.asarray(att, dtype=np.float32)
    bias = np.asarray(bias, dtype=np.float32)

    srcT, cpt, cbase, nchunk, perm = _host_prep(src, dst)
    nc = _build_program(nchunk, cpt, cbase)
    in_maps = _make_in_maps(x, W_l, b_l, W_r, b_r, att, bias, srcT, perm)

    from concourse import bass2jax
    results = bass2jax.run_bass_via_pjrt(nc, in_maps, n_cores=N_CORES)

    out = np.empty((N_NODES, F), dtype=np.float32)
    for k in range(N_CORES):
        pk = perm[k]
        real = pk >= 0
        out[pk[real]] = results[k]["out"][real]
    return out


# revision 3
# speedup vs baseline: 1.4633x; 1.0541x over previous
"""GATv2 message passing (nn_KG_GNN_84430467105347) on 8 Trainium2 NeuronCores.

v3 (dst-sharded, slot-aligned edges, no collectives):
  - Host: self-loops handled as a dedicated first chunk per tile (served from
    SBUF, no gather). Remaining edges dealt to (tile, slot) positions in
    descending in-degree order (~1% padding). Edge (p, c): partition p = dst
    slot.
  - Phase 1: xl table (fp16) for all nodes -> DRAM from host-pretransposed xT;
    poison row at index NPAD kills padded edges' scores (exp -> 0).
    xr + xl for the core's local nodes -> SBUF.
  - Phase 2 per chunk: indirect-DMA gather of xl[src] rows (slot-aligned);
    m = g + xr[tile] on vector; Prelu on scalar; scores = reduce(m*att) on
    vector; exp on scalar; payload = g * p on vector; scatter-add is an
    identity-stationary matmul accumulating [128 dst x (F num | H den)] in
    PSUM. Tile end: divide, add bias, DMA out. Host unpermutes rows.
"""
import sys
sys.path.insert(0, '/opt/trn_rl_repo')
import numpy as np

N_NODES = 100000
IN_DIM = 128
H, C = 4, 32
F = 128
P = 128
NEG_SLOPE = 0.2
N_CORES = 8
NT = 98
NPC = NT * P
NPAD = N_CORES * NPC
NXT = NPAD // P
B = 8             # chunks per edge-phase group
PB = 16           # projection tiles per iteration (xl pass)
PBL = 8           # projection tiles per iteration (local pass)
POISON_K = 30.0


def _host_prep(src, dst):
    """Self-loop chunk per tile + degree-balanced slot layout for real edges."""
    N = N_NODES
    s = np.concatenate([np.asarray(src, dtype=np.int64)]).astype(np.int32)
    d = np.concatenate([np.asarray(dst, dtype=np.int64)]).astype(np.int32)
    deg = np.bincount(d, minlength=N).astype(np.int64)   # excl self loops

    perm = np.full((N_CORES, NPC), -1, dtype=np.int64)
    tile_of = np.zeros(N, dtype=np.int32)
    slot_of = np.zeros(N, dtype=np.int32)
    cpt_k = np.zeros((N_CORES, NT), dtype=np.int64)
    for k in range(N_CORES):
        lo = k * NPC
        hi = min(lo + NPC, N)
        nodes = np.arange(lo, hi, dtype=np.int64)
        order = np.argsort(-deg[nodes], kind='stable')
        pn = nodes[order]
        perm[k, :len(pn)] = pn
        r = np.arange(len(pn))
        tile_of[pn] = r // P
        slot_of[pn] = r % P
        dsorted = np.zeros(NPC, dtype=np.int64)
        dsorted[:len(pn)] = deg[pn]
        cpt_k[k] = dsorted[::P][:NT]

    cpt = cpt_k.max(axis=0) + 1                     # +1 = self-loop chunk
    nchunk = int(cpt.sum())
    pad = (-nchunk) % B
    cpt[NT - 1] += pad
    nchunk += pad
    cbase = np.zeros(NT + 1, dtype=np.int64)
    np.cumsum(cpt, out=cbase[1:])

    srcT = np.full((N_CORES, P, nchunk), NPAD, dtype=np.int32)  # default: poison
    core = d // NPC
    for k in range(N_CORES):
        m = core == k
        dn, sn = d[m], s[m]
        key = tile_of[dn].astype(np.int64) * P + slot_of[dn]
        o2 = np.argsort(key, kind='stable')
        key_s, sn_s = key[o2], sn[o2]
        first = np.searchsorted(key_s, np.arange(NPC))
        occ = np.arange(len(key_s)) - first[key_s]
        chunkpos = cbase[key_s // P] + 1 + occ      # +1 skips self chunk
        srcT[k, key_s % P, chunkpos] = sn_s
    return srcT, cpt, cbase, nchunk, perm


def _build_program(nchunk, cpt, cbase):
    import concourse.bass as bass
    import concourse.mybir as mybir
    import concourse.tile as tile
    from concourse import bacc
    from concourse.masks import make_identity

    f16 = mybir.dt.float16
    f32 = mybir.dt.float32

    t_of = np.zeros(nchunk, dtype=np.int64)
    for t in range(NT):
        t_of[cbase[t]:cbase[t + 1]] = t
    is_self = np.zeros(nchunk, dtype=bool)
    is_self[cbase[:NT]] = True

    nc = bacc.Bacc(None, target_bir_lowering=False)
    xT_in = nc.dram_tensor("xT", [P, NPAD], f16, kind="ExternalInput")
    xlocT_in = nc.dram_tensor("xlocT", [P, NPC], f16, kind="ExternalInput")
    wl_in = nc.dram_tensor("wl", [IN_DIM, F], f16, kind="ExternalInput")
    wr_in = nc.dram_tensor("wr", [IN_DIM, F], f16, kind="ExternalInput")
    blrb_in = nc.dram_tensor("blrb", [P, 2 * F], f16, kind="ExternalInput")
    att_big_in = nc.dram_tensor("att_big", [P, B * F], f16, kind="ExternalInput")
    bias_b_in = nc.dram_tensor("bias_b", [P, F], f32, kind="ExternalInput")
    poison_in = nc.dram_tensor("poison", [1, F], f16, kind="ExternalInput")
    srcT_in = nc.dram_tensor("srcT", [P, nchunk], mybir.dt.int32, kind="ExternalInput")
    out_dram = nc.dram_tensor("out", [NPC, F], f32, kind="ExternalOutput")

    with tile.TileContext(nc) as tc:
        with tc.tile_pool(name="persist", bufs=1) as pp, \
             tc.tile_pool(name="dram", bufs=1, space="DRAM") as dramp:
            xl_dram = dramp.tile([NPAD + 1, F], f16)
            ident = pp.tile([P, P], f16)
            make_identity(nc, ident[:])
            wl = pp.tile([IN_DIM, F], f16)
            nc.sync.dma_start(wl[:], wl_in[:])
            wr = pp.tile([IN_DIM, F], f16)
            nc.sync.dma_start(wr[:], wr_in[:])
            blrb = pp.tile([P, 2 * F], f16)
            nc.sync.dma_start(blrb[:], blrb_in[:])
            att_big = pp.tile([P, B * F], f16)
            nc.sync.dma_start(att_big[:], att_big_in[:])
            bias_b = pp.tile([P, F], f32)
            nc.sync.dma_start(bias_b[:], bias_b_in[:])
            srcT = pp.tile([P, nchunk], mybir.dt.int32)
            nc.sync.dma_start(srcT[:], srcT_in[:])
            nc.sync.dma_start(xl_dram[NPAD:NPAD + 1, :], poison_in[:])
            xr_all = pp.tile([P, NT, F], f16)
            xl_self = pp.tile([P, NT, F], f16)

            # ---------------- phase 1: projections ----------------
            with tc.tile_pool(name="proj_sb", bufs=3) as sb:
              with tc.tile_pool(name="proj_ps", bufs=2, space="PSUM") as ps:
                for i in range(NXT // PB):
                    r0 = i * PB * P
                    xTt = sb.tile([P, PB * P], f16, tag="xTt")
                    nc.sync.dma_start(xTt[:], xT_in[:, r0:r0 + PB * P])
                    prj = ps.tile([P, PB, F], f32, tag="prj", space="PSUM")
                    for j in range(PB):
                        nc.tensor.matmul(out=prj[:, j, :],
                                         lhsT=xTt[:, j * P:(j + 1) * P],
                                         rhs=wl[:], start=True, stop=True)
                    xl_t = sb.tile([P, PB, F], f16, tag="xl_t")
                    nc.vector.tensor_tensor(
                        out=xl_t[:], in0=prj[:],
                        in1=blrb[:, None, :F].to_broadcast([P, PB, F]),
                        op=mybir.AluOpType.add)
                    nc.sync.dma_start(
                        out=xl_dram[r0:r0 + PB * P, :].rearrange(
                            "(b p) f -> p b f", p=P),
                        in_=xl_t[:])
                # local tiles: xr and xl_self from xlocT
              with tc.tile_pool(name="projl_ps", bufs=2, space="PSUM") as ps:
                for i in range((NT + PBL - 1) // PBL):
                    t0 = i * PBL
                    nb = min(PBL, NT - t0)
                    r0 = t0 * P
                    xTt = sb.tile([P, PBL * P], f16, tag="xTt2")
                    nc.sync.dma_start(xTt[:, :nb * P], xlocT_in[:, r0:r0 + nb * P])
                    prj = ps.tile([P, PBL, F], f32, tag="prj", space="PSUM")
                    prl = ps.tile([P, PBL, F], f32, tag="prl", space="PSUM")
                    for j in range(nb):
                        nc.tensor.matmul(out=prj[:, j, :],
                                         lhsT=xTt[:, j * P:(j + 1) * P],
                                         rhs=wr[:], start=True, stop=True)
                        nc.tensor.matmul(out=prl[:, j, :],
                                         lhsT=xTt[:, j * P:(j + 1) * P],
                                         rhs=wl[:], start=True, stop=True)
                    nc.vector.tensor_tensor(
                        out=xr_all[:, t0:t0 + nb, :], in0=prj[:, :nb, :],
                        in1=blrb[:, None, F:].to_broadcast([P, nb, F]),
                        op=mybir.AluOpType.add)
                    nc.vector.tensor_tensor(
                        out=xl_self[:, t0:t0 + nb, :], in0=prl[:, :nb, :],
                        in1=blrb[:, None, :F].to_broadcast([P, nb, F]),
                        op=mybir.AluOpType.add)

            # ---------------- phase 2: edges ----------------
            with tc.tile_pool(name="eg_sb", bufs=6) as sb, \
                 tc.tile_pool(name="out_ps", bufs=2, space="PSUM") as ops, \
                 tc.tile_pool(name="out_sb", bufs=3) as osb:
                ngroups = nchunk // B
                out_ps = None
                for g in range(ngroups):
                    c0 = g * B
                    g_t = sb.tile([P, B, F], f16, tag="g")
                    for j in range(B):
                        c = c0 + j
                        if is_self[c]:
                            continue
                        nc.gpsimd.indirect_dma_start(
                            out=g_t[:, j, :], out_offset=None, in_=xl_dram[:],
                            in_offset=bass.IndirectOffsetOnAxis(
                                ap=srcT[:, c:c + 1], axis=0))
                    # m = xl[src] + xr[dst tile]  (vector; runs of equal tile)
                    m_sb = sb.tile([P, B, F], f16, tag="m")
                    j = 0
                    while j < B:
                        c = c0 + j
                        t = int(t_of[c])
                        if is_self[c]:
                            nc.vector.tensor_tensor(
                                out=m_sb[:, j, :], in0=xl_self[:, t, :],
                                in1=xr_all[:, t, :], op=mybir.AluOpType.add)
                            j += 1
                            continue
                        j1 = j
                        while (j1 < B and int(t_of[c0 + j1]) == t
                               and not is_self[c0 + j1]):
                            j1 += 1
                        nc.vector.tensor_tensor(
                            out=m_sb[:, j:j1, :], in0=g_t[:, j:j1, :],
                            in1=xr_all[:, t, None, :].to_broadcast([P, j1 - j, F]),
                            op=mybir.AluOpType.add)
                        j = j1
                    # LeakyReLU
                    mp16 = sb.tile([P, B, F], f16, tag="mp")
                    nc.scalar.activation(out=mp16[:].rearrange("p b f -> p (b f)"),
                                         in_=m_sb[:].rearrange("p b f -> p (b f)"),
                                         func=mybir.ActivationFunctionType.Prelu,
                                         alpha=NEG_SLOPE)
                    # scores
                    mw = sb.tile([P, B, F], f16, tag="mw")
                    nc.vector.tensor_tensor(
                        out=mw[:].rearrange("p b f -> p (b f)"),
                        in0=mp16[:].rearrange("p b f -> p (b f)"),
                        in1=att_big[:], op=mybir.AluOpType.mult)
                    esc = sb.tile([P, B, H], f32, tag="esc")
                    nc.vector.tensor_reduce(
                        out=esc[:].rearrange("p b h -> p (b h)"),
                        in_=mw[:].rearrange("p b (h c) -> p (b h) c", h=H),
                        axis=mybir.AxisListType.X, op=mybir.AluOpType.add)
                    rhsw = sb.tile([P, B, F + H], f16, tag="rhsw")
                    nc.scalar.activation(out=rhsw[:, :, F:], in_=esc[:],
                                         func=mybir.ActivationFunctionType.Exp)
                    # weighted payload (self chunks use xl_self)
                    j = 0
                    while j < B:
                        c = c0 + j
                        t = int(t_of[c])
                        if is_self[c]:
                            nc.vector.tensor_tensor(
                                out=rhsw[:, j, :F].rearrange(
                                    "p (h c) -> p h c", h=H),
                                in0=xl_self[:, t, :].rearrange(
                                    "p (h c) -> p h c", h=H),
                                in1=rhsw[:, j, F:][:, :, None].to_broadcast(
                                    [P, H, C]),
                                op=mybir.AluOpType.mult)
                            j += 1
                            continue
                        j1 = j
                        while (j1 < B and not is_self[c0 + j1]):
                            j1 += 1
                        nc.vector.tensor_tensor(
                            out=rhsw[:, j:j1, :F].rearrange(
                                "p b (h c) -> p b h c", h=H),
                            in0=g_t[:, j:j1, :].rearrange(
                                "p b (h c) -> p b h c", h=H),
                            in1=rhsw[:, j:j1, F:][:, :, :, None].to_broadcast(
                                [P, j1 - j, H, C]),
                            op=mybir.AluOpType.mult)
                        j = j1
                    # scatter accumulation (identity stationary)
                    for j in range(B):
                        c = c0 + j
                        t = int(t_of[c])
                        if c == cbase[t]:
                            out_ps = ops.tile([P, F + H], f32, tag="out",
                                              space="PSUM")
                        nc.tensor.matmul(out=out_ps[:], lhsT=ident[:],
                                         rhs=rhsw[:, j, :],
                                         start=(c == cbase[t]),
                                         stop=(c == cbase[t + 1] - 1))
                        if c == cbase[t + 1] - 1:
                            den = osb.tile([P, H], f32, tag="den")
                            nc.vector.tensor_scalar_max(den[:], out_ps[:, F:],
                                                        1e-30)
                            recip = osb.tile([P, H], f32, tag="recip")
                            nc.vector.reciprocal(recip[:], den[:])
                            fin = osb.tile([P, F], f32, tag="fin")
                            nc.vector.tensor_tensor(
                                out=fin[:].rearrange("p (h c) -> p h c", h=H),
                                in0=out_ps[:, :F].rearrange(
                                    "p (h c) -> p h c", h=H),
                                in1=recip[:, :, None].to_broadcast([P, H, C]),
                                op=mybir.AluOpType.mult)
                            fin2 = osb.tile([P, F], f32, tag="fin2")
                            nc.vector.tensor_tensor(
                                out=fin2[:], in0=fin[:], in1=bias_b[:],
                                op=mybir.AluOpType.add)
                            nc.sync.dma_start(out_dram[t * P:(t + 1) * P, :],
                                              fin2[:])
    nc.compile()
    return nc


def _make_in_maps(x, W_l, b_l, W_r, b_r, att, bias, srcT, perm):
    f16 = np.float16
    x_pad = np.zeros((NPAD, IN_DIM), dtype=f16)
    x_pad[:N_NODES] = x.astype(f16)
    xT = np.ascontiguousarray(x_pad.T)
    wl = W_l.astype(f16)
    wr = W_r.astype(f16)
    blrb = np.tile(np.concatenate([b_l, b_r])[None, :], (P, 1)).astype(f16)
    att_big = np.tile(att.reshape(1, F), (P, B)).astype(f16)
    bias_b = np.tile(bias[None, :], (P, 1)).astype(np.float32)
    poison = (-np.sign(att.reshape(1, F)) * POISON_K).astype(f16)
    poison[poison == 0] = POISON_K
    in_maps = []
    for k in range(N_CORES):
        pk = np.maximum(perm[k], 0)
        xlocT = np.ascontiguousarray(x_pad[pk].T)
        in_maps.append({
            "xT": xT, "xlocT": xlocT, "wl": wl, "wr": wr, "blrb": blrb,
            "att_big": att_big, "bias_b": bias_b, "poison": poison,
            "srcT": srcT[k],
        })
    return in_maps


def build_for_bench(inp):
    x = np.asarray(inp['x'], dtype=np.float32)
    srcT, cpt, cbase, nchunk, perm = _host_prep(inp['src'], inp['dst'])
    nc = _build_program(nchunk, cpt, cbase)
    in_maps = _make_in_maps(x, inp['W_l'], inp['b_l'], inp['W_r'], inp['b_r'],
                            inp['att'], inp['bias'], srcT, perm)

    def post(results):
        out = np.empty((N_NODES, F), dtype=np.float32)
        for k in range(N_CORES):
            pk = perm[k]
            real = pk >= 0
            out[pk[real]] = results[k]["out"][real]
        return out
    return nc, in_maps, post


def kernel(x, W_l, b_l, W_r, b_r, att, bias, src, dst):
    x = np.asarray(x, dtype=np.float32)
    W_l = np.asarray(W_l, dtype=np.float32)
    W_r = np.asarray(W_r, dtype=np.float32)
    b_l = np.asarray(b_l, dtype=np.float32)
    b_r = np.asarray(b_r, dtype=np.float32)
    att = np.asarray(att, dtype=np.float32)
    bias = np.asarray(bias, dtype=np.float32)

    srcT, cpt, cbase, nchunk, perm = _host_prep(src, dst)
    nc = _build_program(nchunk, cpt, cbase)
    in_maps = _make_in_maps(x, W_l, b_l, W_r, b_r, att, bias, srcT, perm)

    from concourse import bass2jax
    results = bass2jax.run_bass_via_pjrt(nc, in_maps, n_cores=N_CORES)

    out = np.empty((N_NODES, F), dtype=np.float32)
    for k in range(N_CORES):
        pk = perm[k]
        real = pk >= 0
        out[pk[real]] = results[k]["out"][real]
    return out


# revision 4
# speedup vs baseline: 1.4697x; 1.0044x over previous
"""GATv2 message passing (nn_KG_GNN_84430467105347) on 8 Trainium2 NeuronCores.

v3 (dst-sharded, slot-aligned edges, no collectives):
  - Host: self-loops handled as a dedicated first chunk per tile (served from
    SBUF, no gather). Remaining edges dealt to (tile, slot) positions in
    descending in-degree order (~1% padding). Edge (p, c): partition p = dst
    slot.
  - Phase 1: xl table (fp16) for all nodes -> DRAM from host-pretransposed xT;
    poison row at index NPAD kills padded edges' scores (exp -> 0).
    xr + xl for the core's local nodes -> SBUF.
  - Phase 2 per chunk: indirect-DMA gather of xl[src] rows (slot-aligned);
    m = g + xr[tile] on vector; Prelu on scalar; scores = reduce(m*att) on
    vector; exp on scalar; payload = g * p on vector; scatter-add is an
    identity-stationary matmul accumulating [128 dst x (F num | H den)] in
    PSUM. Tile end: divide, add bias, DMA out. Host unpermutes rows.
"""
import sys
sys.path.insert(0, '/opt/trn_rl_repo')
import numpy as np

N_NODES = 100000
IN_DIM = 128
H, C = 4, 32
F = 128
P = 128
NEG_SLOPE = 0.2
N_CORES = 8
NT = 98
NPC = NT * P
NPAD = N_CORES * NPC
NXT = NPAD // P
B = 8             # chunks per edge-phase group
PB = 16           # projection tiles per iteration (xl pass)
PBL = 4           # projection tiles per iteration (local pass)
POISON_K = 30.0
EARLY_G = 32
PAD_BIAS = -60.0


def _host_prep(src, dst):
    """Self-loop chunk per tile + degree-balanced slot layout for real edges."""
    N = N_NODES
    s = np.concatenate([np.asarray(src, dtype=np.int64)]).astype(np.int32)
    d = np.concatenate([np.asarray(dst, dtype=np.int64)]).astype(np.int32)
    deg = np.bincount(d, minlength=N).astype(np.int64)   # excl self loops

    perm = np.full((N_CORES, NPC), -1, dtype=np.int64)
    tile_of = np.zeros(N, dtype=np.int32)
    slot_of = np.zeros(N, dtype=np.int32)
    cpt_k = np.zeros((N_CORES, NT), dtype=np.int64)
    for k in range(N_CORES):
        lo = k * NPC
        hi = min(lo + NPC, N)
        nodes = np.arange(lo, hi, dtype=np.int64)
        order = np.argsort(-deg[nodes], kind='stable')
        pn = nodes[order]
        perm[k, :len(pn)] = pn
        r = np.arange(len(pn))
        tile_of[pn] = r // P
        slot_of[pn] = r % P
        dsorted = np.zeros(NPC, dtype=np.int64)
        dsorted[:len(pn)] = deg[pn]
        cpt_k[k] = dsorted[::P][:NT]

    cpt = cpt_k.max(axis=0) + 1                     # +1 = self-loop chunk
    nchunk = int(cpt.sum())
    pad = (-nchunk) % B
    cpt[NT - 1] += pad
    nchunk += pad
    cbase = np.zeros(NT + 1, dtype=np.int64)
    np.cumsum(cpt, out=cbase[1:])

    srcT = np.full((N_CORES, P, nchunk), NPAD, dtype=np.int32)  # default: poison
    padb = np.full((N_CORES, P, nchunk), PAD_BIAS, dtype=np.float32)
    core = d // NPC
    for k in range(N_CORES):
        m = core == k
        dn, sn = d[m], s[m]
        key = tile_of[dn].astype(np.int64) * P + slot_of[dn]
        o2 = np.argsort(key, kind='stable')
        key_s, sn_s = key[o2], sn[o2]
        first = np.searchsorted(key_s, np.arange(NPC))
        occ = np.arange(len(key_s)) - first[key_s]
        chunkpos = cbase[key_s // P] + 1 + occ      # +1 skips self chunk
        srcT[k, key_s % P, chunkpos] = sn_s
        padb[k, key_s % P, chunkpos] = 0.0
        padb[k, :, cbase[:NT]] = 0.0                # self chunks valid
    return srcT, padb, cpt, cbase, nchunk, perm


def _build_program(nchunk, cpt, cbase):
    import concourse.bass as bass
    import concourse.mybir as mybir
    import concourse.tile as tile
    from concourse import bacc
    from concourse.masks import make_identity

    f16 = mybir.dt.float16
    f32 = mybir.dt.float32

    t_of = np.zeros(nchunk, dtype=np.int64)
    for t in range(NT):
        t_of[cbase[t]:cbase[t + 1]] = t
    is_self = np.zeros(nchunk, dtype=bool)
    is_self[cbase[:NT]] = True

    nc = bacc.Bacc(None, target_bir_lowering=False)
    xT_in = nc.dram_tensor("xT", [P, NPAD], f16, kind="ExternalInput")
    x16_in = nc.dram_tensor("x16", [NPAD + 1, F], f16, kind="ExternalInput")
    xlocT_in = nc.dram_tensor("xlocT", [P, NPC], f16, kind="ExternalInput")
    wl_in = nc.dram_tensor("wl", [IN_DIM, F], f16, kind="ExternalInput")
    wr_in = nc.dram_tensor("wr", [IN_DIM, F], f16, kind="ExternalInput")
    blrb_in = nc.dram_tensor("blrb", [P, 2 * F], f16, kind="ExternalInput")
    att_big_in = nc.dram_tensor("att_big", [P, B * F], f16, kind="ExternalInput")
    bias_b_in = nc.dram_tensor("bias_b", [P, F], f32, kind="ExternalInput")
    poison_in = nc.dram_tensor("poison", [1, F], f16, kind="ExternalInput")
    srcT_in = nc.dram_tensor("srcT", [P, nchunk], mybir.dt.int32, kind="ExternalInput")
    padb_in = nc.dram_tensor("padb", [P, nchunk], f32, kind="ExternalInput")
    out_dram = nc.dram_tensor("out", [NPC, F], f32, kind="ExternalOutput")

    with tile.TileContext(nc) as tc:
        with tc.tile_pool(name="persist", bufs=1) as pp, \
             tc.tile_pool(name="dram", bufs=1, space="DRAM") as dramp, \
             tc.tile_pool(name="out_ps", bufs=2, space="PSUM") as ops, \
             tc.tile_pool(name="tr_ps", bufs=2, space="PSUM") as trp, \
             tc.tile_pool(name="xlp_ps", bufs=2, space="PSUM") as xpp, \
             tc.tile_pool(name="eg_sb", bufs=6) as sb, \
             tc.tile_pool(name="out_sb", bufs=3) as osb:
            xl_dram = dramp.tile([NPAD + 1, F], f16)
            ident = pp.tile([P, P], f16)
            make_identity(nc, ident[:])
            wl = pp.tile([IN_DIM, F], f16)
            nc.sync.dma_start(wl[:], wl_in[:])
            wr = pp.tile([IN_DIM, F], f16)
            nc.sync.dma_start(wr[:], wr_in[:])
            blrb = pp.tile([P, 2 * F], f16)
            nc.sync.dma_start(blrb[:], blrb_in[:])
            att_big = pp.tile([P, B * F], f16)
            nc.sync.dma_start(att_big[:], att_big_in[:])
            bias_b = pp.tile([P, F], f32)
            nc.sync.dma_start(bias_b[:], bias_b_in[:])
            srcT = pp.tile([P, nchunk], mybir.dt.int32)
            nc.sync.dma_start(srcT[:], srcT_in[:])
            padb = pp.tile([P, nchunk], f32)
            nc.sync.dma_start(padb[:], padb_in[:])
            nc.sync.dma_start(xl_dram[NPAD:NPAD + 1, :], poison_in[:])
            xr_all = pp.tile([P, NT, F], f16)
            xl_self = pp.tile([P, NT, F], f16)

            # ---- phase 1a: local tiles (xr, xl_self) -- emitted first ----
            with tc.tile_pool(name="projl_ps", bufs=1, space="PSUM") as psl:
                for i in range((NT + PBL - 1) // PBL):
                    t0 = i * PBL
                    nb = min(PBL, NT - t0)
                    r0 = t0 * P
                    xTt = sb.tile([P, PBL * P], f16, tag="xTt2")
                    nc.sync.dma_start(xTt[:, :nb * P], xlocT_in[:, r0:r0 + nb * P])
                    prj = psl.tile([P, PBL, 2 * F], f32, tag="prjl", space="PSUM")
                    for j in range(nb):
                        nc.tensor.matmul(out=prj[:, j, :F],
                                         lhsT=xTt[:, j * P:(j + 1) * P],
                                         rhs=wr[:], start=True, stop=True)
                        nc.tensor.matmul(out=prj[:, j, F:],
                                         lhsT=xTt[:, j * P:(j + 1) * P],
                                         rhs=wl[:], start=True, stop=True)
                    nc.vector.tensor_tensor(
                        out=xr_all[:, t0:t0 + nb, :], in0=prj[:, :nb, :F],
                        in1=blrb[:, None, F:].to_broadcast([P, nb, F]),
                        op=mybir.AluOpType.add)
                    nc.scalar.copy(xl_self[:, t0:t0 + nb, :], prj[:, :nb, F:])

            # ---- phase 1b: xl table ----
            with tc.tile_pool(name="proj_ps", bufs=1, space="PSUM") as psx:
                for i in range(NXT // PB):
                    r0 = i * PB * P
                    xTt = sb.tile([P, PB * P], f16, tag="xTt")
                    nc.sync.dma_start(xTt[:], xT_in[:, r0:r0 + PB * P])
                    for h2 in range(2):
                        hb = PB // 2
                        prj = psx.tile([P, hb, F], f32, tag="prj", space="PSUM")
                        for j in range(hb):
                            jj = h2 * hb + j
                            nc.tensor.matmul(out=prj[:, j, :],
                                             lhsT=xTt[:, jj * P:(jj + 1) * P],
                                             rhs=wl[:], start=True, stop=True)
                        xl_t = sb.tile([P, hb, F], f16, tag="xl_t")
                        nc.scalar.copy(xl_t[:], prj[:])
                        rr = r0 + h2 * hb * P
                        nc.sync.dma_start(
                            out=xl_dram[rr:rr + hb * P, :].rearrange(
                                "(b p) f -> p b f", p=P),
                            in_=xl_t[:])

            # ---- phase 2: edges ----
            ngroups = nchunk // B
            out_ps = None
            for g in range(ngroups):
                early = g < EARLY_G
                c0 = g * B
                g_t = sb.tile([P, B, F], f16, tag="g")
                for j in range(B):
                    c = c0 + j
                    if is_self[c]:
                        continue
                    nc.gpsimd.indirect_dma_start(
                        out=g_t[:, j, :], out_offset=None,
                        in_=(x16_in[:] if early else xl_dram[:]),
                        in_offset=bass.IndirectOffsetOnAxis(
                            ap=srcT[:, c:c + 1], axis=0))
                if early:
                    # project gathered raw x rows on-chip: xlg = x@W_l + b_l
                    xlg = sb.tile([P, B, F], f16, tag="xlg")
                    for j in range(B):
                        c = c0 + j
                        if is_self[c]:
                            continue
                        tr = trp.tile([P, P], f16, tag="tr", space="PSUM")
                        nc.tensor.transpose(out=tr[:], in_=g_t[:, j, :],
                                            identity=ident[:])
                        xgT = sb.tile([P, P], f16, tag="xgT")
                        nc.vector.tensor_copy(xgT[:], tr[:])
                        xlp = xpp.tile([P, F], f32, tag="xlp", space="PSUM")
                        nc.tensor.matmul(out=xlp[:], lhsT=xgT[:], rhs=wl[:],
                                         start=True, stop=True)
                        nc.scalar.copy(xlg[:, j, :], xlp[:])
                    gsrc = xlg
                else:
                    gsrc = g_t
                # m = xl[src] + xr[dst tile]  (vector; runs of equal tile)
                m_sb = sb.tile([P, B, F], f16, tag="m")
                j = 0
                while j < B:
                    c = c0 + j
                    t = int(t_of[c])
                    if is_self[c]:
                        nc.vector.tensor_tensor(
                            out=m_sb[:, j, :], in0=xl_self[:, t, :],
                            in1=xr_all[:, t, :], op=mybir.AluOpType.add)
                        j += 1
                        continue
                    j1 = j
                    while (j1 < B and int(t_of[c0 + j1]) == t
                           and not is_self[c0 + j1]):
                        j1 += 1
                    nc.vector.tensor_tensor(
                        out=m_sb[:, j:j1, :], in0=gsrc[:, j:j1, :],
                        in1=xr_all[:, t, None, :].to_broadcast([P, j1 - j, F]),
                        op=mybir.AluOpType.add)
                    j = j1
                # LeakyReLU
                mp16 = sb.tile([P, B, F], f16, tag="mp")
                nc.scalar.activation(out=mp16[:].rearrange("p b f -> p (b f)"),
                                     in_=m_sb[:].rearrange("p b f -> p (b f)"),
                                     func=mybir.ActivationFunctionType.Prelu,
                                     alpha=NEG_SLOPE)
                # scores
                mw = sb.tile([P, B, F], f16, tag="mw")
                nc.vector.tensor_tensor(
                    out=mw[:].rearrange("p b f -> p (b f)"),
                    in0=mp16[:].rearrange("p b f -> p (b f)"),
                    in1=att_big[:], op=mybir.AluOpType.mult)
                esc = sb.tile([P, B, H], f32, tag="esc")
                nc.vector.tensor_reduce(
                    out=esc[:].rearrange("p b h -> p (b h)"),
                    in_=mw[:].rearrange("p b (h c) -> p (b h) c", h=H),
                    axis=mybir.AxisListType.X, op=mybir.AluOpType.add)
                if early:
                    esc2 = sb.tile([P, B, H], f32, tag="esc2")
                    nc.vector.tensor_tensor(
                        out=esc2[:], in0=esc[:],
                        in1=padb[:, c0:c0 + B, None].to_broadcast([P, B, H]),
                        op=mybir.AluOpType.add)
                else:
                    esc2 = esc
                rhsw = sb.tile([P, B, F + H], f16, tag="rhsw")
                nc.scalar.activation(out=rhsw[:, :, F:], in_=esc2[:],
                                     func=mybir.ActivationFunctionType.Exp)
                # weighted payload (self chunks use xl_self)
                j = 0
                while j < B:
                    c = c0 + j
                    t = int(t_of[c])
                    if is_self[c]:
                        nc.vector.tensor_tensor(
                            out=rhsw[:, j, :F].rearrange(
                                "p (h c) -> p h c", h=H),
                            in0=xl_self[:, t, :].rearrange(
                                "p (h c) -> p h c", h=H),
                            in1=rhsw[:, j, F:][:, :, None].to_broadcast(
                                [P, H, C]),
                            op=mybir.AluOpType.mult)
                        j += 1
                        continue
                    j1 = j
                    while (j1 < B and not is_self[c0 + j1]):
                        j1 += 1
                    nc.vector.tensor_tensor(
                        out=rhsw[:, j:j1, :F].rearrange(
                            "p b (h c) -> p b h c", h=H),
                        in0=gsrc[:, j:j1, :].rearrange(
                            "p b (h c) -> p b h c", h=H),
                        in1=rhsw[:, j:j1, F:][:, :, :, None].to_broadcast(
                            [P, j1 - j, H, C]),
                        op=mybir.AluOpType.mult)
                    j = j1
                # scatter accumulation (identity stationary)
                for j in range(B):
                    c = c0 + j
                    t = int(t_of[c])
                    if c == cbase[t]:
                        out_ps = ops.tile([P, F + H], f32, tag="out",
                                          space="PSUM")
                    nc.tensor.matmul(out=out_ps[:], lhsT=ident[:],
                                     rhs=rhsw[:, j, :],
                                     start=(c == cbase[t]),
                                     stop=(c == cbase[t + 1] - 1))
                    if c == cbase[t + 1] - 1:
                        den = osb.tile([P, H], f32, tag="den")
                        nc.vector.tensor_scalar_max(den[:], out_ps[:, F:],
                                                    1e-30)
                        recip = osb.tile([P, H], f32, tag="recip")
                        nc.vector.reciprocal(recip[:], den[:])
                        fin = osb.tile([P, F], f32, tag="fin")
                        nc.vector.tensor_tensor(
                            out=fin[:].rearrange("p (h c) -> p h c", h=H),
                            in0=out_ps[:, :F].rearrange(
                                "p (h c) -> p h c", h=H),
                            in1=recip[:, :, None].to_broadcast([P, H, C]),
                            op=mybir.AluOpType.mult)
                        fin2 = osb.tile([P, F], f32, tag="fin2")
                        nc.vector.tensor_tensor(
                            out=fin2[:], in0=fin[:], in1=bias_b[:],
                            op=mybir.AluOpType.add)
                        nc.sync.dma_start(out_dram[t * P:(t + 1) * P, :],
                                          fin2[:])
    nc.compile()
    return nc


def _make_in_maps(x, W_l, b_l, W_r, b_r, att, bias, srcT, padb, perm):
    f16 = np.float16
    x_pad = np.zeros((NPAD, IN_DIM), dtype=f16)
    x_pad[:N_NODES] = x.astype(f16)
    xT = np.ascontiguousarray(x_pad.T)
    x16 = np.zeros((NPAD + 1, IN_DIM), dtype=f16)
    x16[:NPAD] = x_pad
    wl = W_l.astype(f16)
    wr = W_r.astype(f16)
    blrb = np.tile(np.concatenate([b_l, b_l + b_r])[None, :], (P, 1)).astype(f16)
    att_big = np.tile(att.reshape(1, F), (P, B)).astype(f16)
    bias_b = np.tile((bias + b_l)[None, :], (P, 1)).astype(np.float32)
    poison = (-np.sign(att.reshape(1, F)) * POISON_K).astype(f16)
    poison[poison == 0] = POISON_K
    in_maps = []
    for k in range(N_CORES):
        pk = np.maximum(perm[k], 0)
        xlocT = np.ascontiguousarray(x_pad[pk].T)
        in_maps.append({
            "xT": xT, "x16": x16, "xlocT": xlocT, "wl": wl, "wr": wr,
            "blrb": blrb, "att_big": att_big, "bias_b": bias_b,
            "poison": poison, "srcT": srcT[k], "padb": padb[k],
        })
    return in_maps


def build_for_bench(inp):
    x = np.asarray(inp['x'], dtype=np.float32)
    srcT, padb, cpt, cbase, nchunk, perm = _host_prep(inp['src'], inp['dst'])
    nc = _build_program(nchunk, cpt, cbase)
    in_maps = _make_in_maps(x, inp['W_l'], inp['b_l'], inp['W_r'], inp['b_r'],
                            inp['att'], inp['bias'], srcT, padb, perm)

    def post(results):
        out = np.empty((N_NODES, F), dtype=np.float32)
        for k in range(N_CORES):
            pk = perm[k]
            real = pk >= 0
            out[pk[real]] = results[k]["out"][real]
        return out
    return nc, in_maps, post


def kernel(x, W_l, b_l, W_r, b_r, att, bias, src, dst):
    x = np.asarray(x, dtype=np.float32)
    W_l = np.asarray(W_l, dtype=np.float32)
    W_r = np.asarray(W_r, dtype=np.float32)
    b_l = np.asarray(b_l, dtype=np.float32)
    b_r = np.asarray(b_r, dtype=np.float32)
    att = np.asarray(att, dtype=np.float32)
    bias = np.asarray(bias, dtype=np.float32)

    srcT, padb, cpt, cbase, nchunk, perm = _host_prep(src, dst)
    nc = _build_program(nchunk, cpt, cbase)
    in_maps = _make_in_maps(x, W_l, b_l, W_r, b_r, att, bias, srcT, padb, perm)

    from concourse import bass2jax
    results = bass2jax.run_bass_via_pjrt(nc, in_maps, n_cores=N_CORES)

    out = np.empty((N_NODES, F), dtype=np.float32)
    for k in range(N_CORES):
        pk = perm[k]
        real = pk >= 0
        out[pk[real]] = results[k]["out"][real]
    return out


# revision 5
# speedup vs baseline: 1.4707x; 1.0007x over previous
"""GATv2 message passing (nn_KG_GNN_84430467105347) on 8 Trainium2 NeuronCores.

v3 (dst-sharded, slot-aligned edges, no collectives):
  - Host: self-loops handled as a dedicated first chunk per tile (served from
    SBUF, no gather). Remaining edges dealt to (tile, slot) positions in
    descending in-degree order (~1% padding). Edge (p, c): partition p = dst
    slot.
  - Phase 1: xl table (fp16) for all nodes -> DRAM from host-pretransposed xT;
    poison row at index NPAD kills padded edges' scores (exp -> 0).
    xr + xl for the core's local nodes -> SBUF.
  - Phase 2 per chunk: indirect-DMA gather of xl[src] rows (slot-aligned);
    m = g + xr[tile] on vector; Prelu on scalar; scores = reduce(m*att) on
    vector; exp on scalar; payload = g * p on vector; scatter-add is an
    identity-stationary matmul accumulating [128 dst x (F num | H den)] in
    PSUM. Tile end: divide, add bias, DMA out. Host unpermutes rows.
"""
import sys
sys.path.insert(0, '/opt/trn_rl_repo')
import numpy as np

N_NODES = 100000
IN_DIM = 128
H, C = 4, 32
F = 128
P = 128
NEG_SLOPE = 0.2
N_CORES = 8
NT = 98
NPC = NT * P
NPAD = N_CORES * NPC
NXT = NPAD // P
B = 8             # chunks per edge-phase group
PB = 16           # projection tiles per iteration (xl pass)
PBL = 4           # projection tiles per iteration (local pass)
POISON_K = 30.0
EARLY_G = 32
NPRE = 7
PAD_BIAS = -60.0


def _host_prep(src, dst):
    """Self-loop chunk per tile + degree-balanced slot layout for real edges."""
    N = N_NODES
    s = np.concatenate([np.asarray(src, dtype=np.int64)]).astype(np.int32)
    d = np.concatenate([np.asarray(dst, dtype=np.int64)]).astype(np.int32)
    deg = np.bincount(d, minlength=N).astype(np.int64)   # excl self loops

    perm = np.full((N_CORES, NPC), -1, dtype=np.int64)
    tile_of = np.zeros(N, dtype=np.int32)
    slot_of = np.zeros(N, dtype=np.int32)
    cpt_k = np.zeros((N_CORES, NT), dtype=np.int64)
    for k in range(N_CORES):
        lo = k * NPC
        hi = min(lo + NPC, N)
        nodes = np.arange(lo, hi, dtype=np.int64)
        order = np.argsort(-deg[nodes], kind='stable')
        pn = nodes[order]
        perm[k, :len(pn)] = pn
        r = np.arange(len(pn))
        tile_of[pn] = r // P
        slot_of[pn] = r % P
        dsorted = np.zeros(NPC, dtype=np.int64)
        dsorted[:len(pn)] = deg[pn]
        cpt_k[k] = dsorted[::P][:NT]

    cpt = cpt_k.max(axis=0) + 1                     # +1 = self-loop chunk
    nchunk = int(cpt.sum())
    pad = (-nchunk) % B
    cpt[NT - 1] += pad
    nchunk += pad
    cbase = np.zeros(NT + 1, dtype=np.int64)
    np.cumsum(cpt, out=cbase[1:])

    srcT = np.full((N_CORES, P, nchunk), NPAD, dtype=np.int32)  # default: poison
    padb = np.full((N_CORES, P, nchunk), PAD_BIAS, dtype=np.float32)
    core = d // NPC
    for k in range(N_CORES):
        m = core == k
        dn, sn = d[m], s[m]
        key = tile_of[dn].astype(np.int64) * P + slot_of[dn]
        o2 = np.argsort(key, kind='stable')
        key_s, sn_s = key[o2], sn[o2]
        first = np.searchsorted(key_s, np.arange(NPC))
        occ = np.arange(len(key_s)) - first[key_s]
        chunkpos = cbase[key_s // P] + 1 + occ      # +1 skips self chunk
        srcT[k, key_s % P, chunkpos] = sn_s
        padb[k, key_s % P, chunkpos] = 0.0
        padb[k, :, cbase[:NT]] = 0.0                # self chunks valid
    return srcT, padb, cpt, cbase, nchunk, perm


def _build_program(nchunk, cpt, cbase):
    import concourse.bass as bass
    import concourse.mybir as mybir
    import concourse.tile as tile
    from concourse import bacc
    from concourse.masks import make_identity

    f16 = mybir.dt.float16
    f32 = mybir.dt.float32

    t_of = np.zeros(nchunk, dtype=np.int64)
    for t in range(NT):
        t_of[cbase[t]:cbase[t + 1]] = t
    is_self = np.zeros(nchunk, dtype=bool)
    is_self[cbase[:NT]] = True

    nc = bacc.Bacc(None, target_bir_lowering=False)
    xT_in = nc.dram_tensor("xT", [P, NPAD], f16, kind="ExternalInput")
    x16_in = nc.dram_tensor("x16", [NPAD + 1, F], f16, kind="ExternalInput")
    xlocT_in = nc.dram_tensor("xlocT", [P, NPC], f16, kind="ExternalInput")
    wl_in = nc.dram_tensor("wl", [IN_DIM, F], f16, kind="ExternalInput")
    wr_in = nc.dram_tensor("wr", [IN_DIM, F], f16, kind="ExternalInput")
    blrb_in = nc.dram_tensor("blrb", [P, 2 * F], f16, kind="ExternalInput")
    att_big_in = nc.dram_tensor("att_big", [P, B * F], f16, kind="ExternalInput")
    bias_b_in = nc.dram_tensor("bias_b", [P, F], f32, kind="ExternalInput")
    poison_in = nc.dram_tensor("poison", [1, F], f16, kind="ExternalInput")
    srcT_in = nc.dram_tensor("srcT", [P, nchunk], mybir.dt.int32, kind="ExternalInput")
    padb_in = nc.dram_tensor("padb", [P, nchunk], f32, kind="ExternalInput")
    out_dram = nc.dram_tensor("out", [NPC, F], f32, kind="ExternalOutput")

    with tile.TileContext(nc) as tc:
        with tc.tile_pool(name="persist", bufs=1) as pp, \
             tc.tile_pool(name="dram", bufs=1, space="DRAM") as dramp, \
             tc.tile_pool(name="out_ps", bufs=2, space="PSUM") as ops, \
             tc.tile_pool(name="tr_ps", bufs=2, space="PSUM") as trp, \
             tc.tile_pool(name="xlp_ps", bufs=2, space="PSUM") as xpp, \
             tc.tile_pool(name="eg_sb", bufs=6) as sb, \
             tc.tile_pool(name="out_sb", bufs=3) as osb:
            xl_dram = dramp.tile([NPAD + 1, F], f16)
            ident = pp.tile([P, P], f16)
            make_identity(nc, ident[:])
            wl = pp.tile([IN_DIM, F], f16)
            nc.sync.dma_start(wl[:], wl_in[:])
            wr = pp.tile([IN_DIM, F], f16)
            nc.sync.dma_start(wr[:], wr_in[:])
            blrb = pp.tile([P, 2 * F], f16)
            nc.sync.dma_start(blrb[:], blrb_in[:])
            att_big = pp.tile([P, B * F], f16)
            nc.sync.dma_start(att_big[:], att_big_in[:])
            bias_b = pp.tile([P, F], f32)
            nc.sync.dma_start(bias_b[:], bias_b_in[:])
            srcT = pp.tile([P, nchunk], mybir.dt.int32)
            nc.sync.dma_start(srcT[:], srcT_in[:])
            padb = pp.tile([P, nchunk], f32)
            nc.sync.dma_start(padb[:], padb_in[:])
            nc.sync.dma_start(xl_dram[NPAD:NPAD + 1, :], poison_in[:])
            xr_all = pp.tile([P, NT, F], f16)
            xl_self = pp.tile([P, NT, F], f16)

            # prefetch the last NPRE groups' gathers from raw x (no table dep)
            # into persistent tiles: they fill the early->table transition gap
            ngroups = nchunk // B
            pre_first = ngroups - NPRE
            pre_g = []
            for pi in range(NPRE):
                gt = pp.tile([P, B, F], f16, tag=f"pre{pi}")
                pre_g.append(gt)
                c0 = (pre_first + pi) * B
                for j in range(B):
                    c = c0 + j
                    if is_self[c]:
                        continue
                    nc.gpsimd.indirect_dma_start(
                        out=gt[:, j, :], out_offset=None, in_=x16_in[:],
                        in_offset=bass.IndirectOffsetOnAxis(
                            ap=srcT[:, c:c + 1], axis=0))

            # ---- phase 1a: local tiles (xr, xl_self) -- emitted first ----
            with tc.tile_pool(name="projl_ps", bufs=1, space="PSUM") as psl:
                for i in range((NT + PBL - 1) // PBL):
                    t0 = i * PBL
                    nb = min(PBL, NT - t0)
                    r0 = t0 * P
                    xTt = sb.tile([P, PBL * P], f16, tag="xTt2")
                    nc.sync.dma_start(xTt[:, :nb * P], xlocT_in[:, r0:r0 + nb * P])
                    prj = psl.tile([P, PBL, 2 * F], f32, tag="prjl", space="PSUM")
                    for j in range(nb):
                        nc.tensor.matmul(out=prj[:, j, :F],
                                         lhsT=xTt[:, j * P:(j + 1) * P],
                                         rhs=wr[:], start=True, stop=True)
                        nc.tensor.matmul(out=prj[:, j, F:],
                                         lhsT=xTt[:, j * P:(j + 1) * P],
                                         rhs=wl[:], start=True, stop=True)
                    nc.vector.tensor_tensor(
                        out=xr_all[:, t0:t0 + nb, :], in0=prj[:, :nb, :F],
                        in1=blrb[:, None, F:].to_broadcast([P, nb, F]),
                        op=mybir.AluOpType.add)
                    nc.scalar.copy(xl_self[:, t0:t0 + nb, :], prj[:, :nb, F:])

            # ---- phase 1b: xl table ----
            with tc.tile_pool(name="proj_ps", bufs=1, space="PSUM") as psx:
                for i in range(NXT // PB):
                    r0 = i * PB * P
                    xTt = sb.tile([P, PB * P], f16, tag="xTt")
                    nc.sync.dma_start(xTt[:], xT_in[:, r0:r0 + PB * P])
                    for h2 in range(2):
                        hb = PB // 2
                        prj = psx.tile([P, hb, F], f32, tag="prj", space="PSUM")
                        for j in range(hb):
                            jj = h2 * hb + j
                            nc.tensor.matmul(out=prj[:, j, :],
                                             lhsT=xTt[:, jj * P:(jj + 1) * P],
                                             rhs=wl[:], start=True, stop=True)
                        xl_t = sb.tile([P, hb, F], f16, tag="xl_t")
                        nc.scalar.copy(xl_t[:], prj[:])
                        rr = r0 + h2 * hb * P
                        nc.sync.dma_start(
                            out=xl_dram[rr:rr + hb * P, :].rearrange(
                                "(b p) f -> p b f", p=P),
                            in_=xl_t[:])

            # ---- phase 2: edges ----
            out_ps = None
            for g in range(ngroups):
                pre = g >= pre_first
                early = (g < EARLY_G) or pre
                c0 = g * B
                if pre:
                    g_t = pre_g[g - pre_first]
                else:
                    g_t = sb.tile([P, B, F], f16, tag="g")
                    for j in range(B):
                        c = c0 + j
                        if is_self[c]:
                            continue
                        nc.gpsimd.indirect_dma_start(
                            out=g_t[:, j, :], out_offset=None,
                            in_=(x16_in[:] if early else xl_dram[:]),
                            in_offset=bass.IndirectOffsetOnAxis(
                                ap=srcT[:, c:c + 1], axis=0))
                if early:
                    # project gathered raw x rows on-chip: xlg = x@W_l + b_l
                    xlg = sb.tile([P, B, F], f16, tag="xlg")
                    for j in range(B):
                        c = c0 + j
                        if is_self[c]:
                            continue
                        tr = trp.tile([P, P], f16, tag="tr", space="PSUM")
                        nc.tensor.transpose(out=tr[:], in_=g_t[:, j, :],
                                            identity=ident[:])
                        xgT = sb.tile([P, P], f16, tag="xgT")
                        nc.vector.tensor_copy(xgT[:], tr[:])
                        xlp = xpp.tile([P, F], f32, tag="xlp", space="PSUM")
                        nc.tensor.matmul(out=xlp[:], lhsT=xgT[:], rhs=wl[:],
                                         start=True, stop=True)
                        nc.scalar.copy(xlg[:, j, :], xlp[:])
                    gsrc = xlg
                else:
                    gsrc = g_t
                # m = xl[src] + xr[dst tile]  (vector; runs of equal tile)
                m_sb = sb.tile([P, B, F], f16, tag="m")
                j = 0
                while j < B:
                    c = c0 + j
                    t = int(t_of[c])
                    if is_self[c]:
                        nc.vector.tensor_tensor(
                            out=m_sb[:, j, :], in0=xl_self[:, t, :],
                            in1=xr_all[:, t, :], op=mybir.AluOpType.add)
                        j += 1
                        continue
                    j1 = j
                    while (j1 < B and int(t_of[c0 + j1]) == t
                           and not is_self[c0 + j1]):
                        j1 += 1
                    nc.vector.tensor_tensor(
                        out=m_sb[:, j:j1, :], in0=gsrc[:, j:j1, :],
                        in1=xr_all[:, t, None, :].to_broadcast([P, j1 - j, F]),
                        op=mybir.AluOpType.add)
                    j = j1
                # LeakyReLU
                mp16 = sb.tile([P, B, F], f16, tag="mp")
                nc.scalar.activation(out=mp16[:].rearrange("p b f -> p (b f)"),
                                     in_=m_sb[:].rearrange("p b f -> p (b f)"),
                                     func=mybir.ActivationFunctionType.Prelu,
                                     alpha=NEG_SLOPE)
                # scores
                mw = sb.tile([P, B, F], f16, tag="mw")
                nc.vector.tensor_tensor(
                    out=mw[:].rearrange("p b f -> p (b f)"),
                    in0=mp16[:].rearrange("p b f -> p (b f)"),
                    in1=att_big[:], op=mybir.AluOpType.mult)
                esc = sb.tile([P, B, H], f32, tag="esc")
                nc.vector.tensor_reduce(
                    out=esc[:].rearrange("p b h -> p (b h)"),
                    in_=mw[:].rearrange("p b (h c) -> p (b h) c", h=H),
                    axis=mybir.AxisListType.X, op=mybir.AluOpType.add)
                if early:
                    esc2 = sb.tile([P, B, H], f32, tag="esc2")
                    nc.vector.tensor_tensor(
                        out=esc2[:], in0=esc[:],
                        in1=padb[:, c0:c0 + B, None].to_broadcast([P, B, H]),
                        op=mybir.AluOpType.add)
                else:
                    esc2 = esc
                rhsw = sb.tile([P, B, F + H], f16, tag="rhsw")
                nc.scalar.activation(out=rhsw[:, :, F:], in_=esc2[:],
                                     func=mybir.ActivationFunctionType.Exp)
                # weighted payload (self chunks use xl_self)
                j = 0
                while j < B:
                    c = c0 + j
                    t = int(t_of[c])
                    if is_self[c]:
                        nc.vector.tensor_tensor(
                            out=rhsw[:, j, :F].rearrange(
                                "p (h c) -> p h c", h=H),
                            in0=xl_self[:, t, :].rearrange(
                                "p (h c) -> p h c", h=H),
                            in1=rhsw[:, j, F:][:, :, None].to_broadcast(
                                [P, H, C]),
                            op=mybir.AluOpType.mult)
                        j += 1
                        continue
                    j1 = j
                    while (j1 < B and not is_self[c0 + j1]):
                        j1 += 1
                    nc.vector.tensor_tensor(
                        out=rhsw[:, j:j1, :F].rearrange(
                            "p b (h c) -> p b h c", h=H),
                        in0=gsrc[:, j:j1, :].rearrange(
                            "p b (h c) -> p b h c", h=H),
                        in1=rhsw[:, j:j1, F:][:, :, :, None].to_broadcast(
                            [P, j1 - j, H, C]),
                        op=mybir.AluOpType.mult)
                    j = j1
                # scatter accumulation (identity stationary)
                for j in range(B):
                    c = c0 + j
                    t = int(t_of[c])
                    if c == cbase[t]:
                        out_ps = ops.tile([P, F + H], f32, tag="out",
                                          space="PSUM")
                    nc.tensor.matmul(out=out_ps[:], lhsT=ident[:],
                                     rhs=rhsw[:, j, :],
                                     start=(c == cbase[t]),
                                     stop=(c == cbase[t + 1] - 1))
                    if c == cbase[t + 1] - 1:
                        den = osb.tile([P, H], f32, tag="den")
                        nc.vector.tensor_scalar_max(den[:], out_ps[:, F:],
                                                    1e-30)
                        recip = osb.tile([P, H], f32, tag="recip")
                        nc.vector.reciprocal(recip[:], den[:])
                        fin = osb.tile([P, F], f32, tag="fin")
                        nc.vector.tensor_tensor(
                            out=fin[:].rearrange("p (h c) -> p h c", h=H),
                            in0=out_ps[:, :F].rearrange(
                                "p (h c) -> p h c", h=H),
                            in1=recip[:, :, None].to_broadcast([P, H, C]),
                            op=mybir.AluOpType.mult)
                        fin2 = osb.tile([P, F], f32, tag="fin2")
                        nc.vector.tensor_tensor(
                            out=fin2[:], in0=fin[:], in1=bias_b[:],
                            op=mybir.AluOpType.add)
                        nc.sync.dma_start(out_dram[t * P:(t + 1) * P, :],
                                          fin2[:])
    nc.compile()
    return nc


def _make_in_maps(x, W_l, b_l, W_r, b_r, att, bias, srcT, padb, perm):
    f16 = np.float16
    x_pad = np.zeros((NPAD, IN_DIM), dtype=f16)
    x_pad[:N_NODES] = x.astype(f16)
    xT = np.ascontiguousarray(x_pad.T)
    x16 = np.zeros((NPAD + 1, IN_DIM), dtype=f16)
    x16[:NPAD] = x_pad
    wl = W_l.astype(f16)
    wr = W_r.astype(f16)
    blrb = np.tile(np.concatenate([b_l, b_l + b_r])[None, :], (P, 1)).astype(f16)
    att_big = np.tile(att.reshape(1, F), (P, B)).astype(f16)
    bias_b = np.tile((bias + b_l)[None, :], (P, 1)).astype(np.float32)
    poison = (-np.sign(att.reshape(1, F)) * POISON_K).astype(f16)
    poison[poison == 0] = POISON_K
    in_maps = []
    for k in range(N_CORES):
        pk = np.maximum(perm[k], 0)
        xlocT = np.ascontiguousarray(x_pad[pk].T)
        in_maps.append({
            "xT": xT, "x16": x16, "xlocT": xlocT, "wl": wl, "wr": wr,
            "blrb": blrb, "att_big": att_big, "bias_b": bias_b,
            "poison": poison, "srcT": srcT[k], "padb": padb[k],
        })
    return in_maps


def build_for_bench(inp):
    x = np.asarray(inp['x'], dtype=np.float32)
    srcT, padb, cpt, cbase, nchunk, perm = _host_prep(inp['src'], inp['dst'])
    nc = _build_program(nchunk, cpt, cbase)
    in_maps = _make_in_maps(x, inp['W_l'], inp['b_l'], inp['W_r'], inp['b_r'],
                            inp['att'], inp['bias'], srcT, padb, perm)

    def post(results):
        out = np.empty((N_NODES, F), dtype=np.float32)
        for k in range(N_CORES):
            pk = perm[k]
            real = pk >= 0
            out[pk[real]] = results[k]["out"][real]
        return out
    return nc, in_maps, post


def kernel(x, W_l, b_l, W_r, b_r, att, bias, src, dst):
    x = np.asarray(x, dtype=np.float32)
    W_l = np.asarray(W_l, dtype=np.float32)
    W_r = np.asarray(W_r, dtype=np.float32)
    b_l = np.asarray(b_l, dtype=np.float32)
    b_r = np.asarray(b_r, dtype=np.float32)
    att = np.asarray(att, dtype=np.float32)
    bias = np.asarray(bias, dtype=np.float32)

    srcT, padb, cpt, cbase, nchunk, perm = _host_prep(src, dst)
    nc = _build_program(nchunk, cpt, cbase)
    in_maps = _make_in_maps(x, W_l, b_l, W_r, b_r, att, bias, srcT, padb, perm)

    from concourse import bass2jax
    results = bass2jax.run_bass_via_pjrt(nc, in_maps, n_cores=N_CORES)

    out = np.empty((N_NODES, F), dtype=np.float32)
    for k in range(N_CORES):
        pk = perm[k]
        real = pk >= 0
        out[pk[real]] = results[k]["out"][real]
    return out
